# revision 70
# baseline (speedup 1.0000x reference)
"""BiLSTM tagger kernel for 8 Trainium2 NeuronCores.

Model (per reference): x = emb[tokens]; h_f = LSTM_f(x); h_b = LSTM_b(rev(x));
probs = softmax([h_f, h_b] @ Wd + bd).

Sharding: data-parallel over batch.  Each of the 8 cores handles 32 sequences
and runs BOTH directions for them, so no cross-core communication is needed;
the host shards tokens and concatenates outputs.

Per-core layout ("transposed" LSTM): the feature dim lives on SBUF partitions
and the 32 sequences on the free dim.  Token slot s = seq + 32*t.
 - host gathers emb rows into slot layout, ships them bf16; PE transposes
   into xT [128 (E-slice), kt, slots] just-in-time, one chunk ahead of use
 - recurrence (per direction, 128 steps): ONE PSUM accumulation group per
   step computes z = W^T x_t + U^T h_{t-1} directly (the x-projection matmuls
   are h-independent, so the in-order PE queue runs them ahead; there is no
   separate projection pass or xz buffer).  Exactly one start=True per group:
   a start-matmul clears has_written for the whole PSUM bank, so per-region
   starts would corrupt later accumulates.
 - gates in keras order [i, f, g, o] with g's z pre-scaled x2: the main
   sigmoid covers (i,f,g) and fires off just those 12 U-matmuls; the o-gate
   matmuls + sigmoid follow off the critical path.
 - cell update in 3 DVE ops on a half-scale cell C = c/2:
   t1=(s-0.5)*i (fused STT), fc=f*C, C=fc+t1; tanh uses the free input
   scale=2.  h is written per k-tile so next-step U matmuls start earlier.
 - the two direction chains interleave per step with the ACT queue ordered
   [sigA_f, sigA_b, sigB_f, tanh_f, sigB_b, tanh_b] to match execution order
   (engines issue in-order; a misplaced instruction head-blocks the queue).
 - dense runs every 8 steps from both directions; softmax+store is emitted
   per j-pair as soon as both directions have covered it (deferred two pairs
   so it never delays the recurrence chain).

Weights are marshalled host-side into the exact SBUF tile layouts (k-tile on
partitions) and cast to bf16; PSUM accumulation stays fp32.
"""

import sys

import numpy as np

if "/opt/trn_rl_repo" not in sys.path:
    sys.path.insert(0, "/opt/trn_rl_repo")

V, E, T, H, NTAGS, B = 50000, 256, 128, 256, 17, 256
NCORES = 8
BS = B // NCORES            # sequences per core
P = 128
KT = E // P                 # 2 k-tiles for E and H
M8 = (4 * H) // P           # 8 m-tiles over the gate dim
# The SWDGE indirect-DMA (gather) path is unreliable in this environment
# (works after boot, breaks persistently after any device fault), so the
# embedding rows are gathered host-side into the slot layout and streamed
# to the device as a regular input.  Device work is otherwise identical.
USE_HOST_GATHER = True
SKEW = 0
ALT = 0
GBUFS = 3
CELL_BF16 = True

_CACHE = {}


def _legalize_waits(nc):
    """TRN2 hw instructions have one semaphore-wait slot; Tile can attach
    several.  Split extras onto same-engine NOPs placed just before."""
    import concourse.mybir as mybir

    for _, bbb in nc.bb_map.items():
        bb = bbb.bb
        new = []
        for inst in bb.instructions:
            si = inst.sync_info
            waits = list(si.on_wait) if (si and si.on_wait) else []
            if len(waits) > 1:
                for k, w in enumerate(waits[:-1]):
                    nop = mybir.InstNoOp(
                        name=f"{inst.name}_lw{k}",
                        engine=inst.engine,
                        sync_info=mybir.SyncInfo(on_wait=[w], on_update=[]),
                        bass_nofuse=True,
                    )
                    nc.register_instruction(nop)
                    new.append(nop)
                inst.sync_info = mybir.SyncInfo(
                    on_wait=[waits[-1]],
                    on_update=list(si.on_update) if si.on_update else [],
                )
            new.append(inst)
        bb.instructions = new


def build_program(t_len=T, vocab=V, no_bias=False):
    """Build the per-core SPMD program.  t_len must be a multiple of 16."""
    from contextlib import ExitStack

    import concourse.bass as bass
    import concourse.mybir as mybir
    import concourse.tile as tile
    from concourse.masks import make_identity

    f32 = mybir.dt.float32
    bf16 = mybir.dt.bfloat16
    SIG = mybir.ActivationFunctionType.Sigmoid
    TANH = mybir.ActivationFunctionType.Tanh
    EXP = mybir.ActivationFunctionType.Exp
    MUL = mybir.AluOpType.mult
    ADD = mybir.AluOpType.add

    CDT = bf16 if CELL_BF16 else f32
    SLOTS = BS * t_len
    JT = SLOTS // P             # 128-slot tiles (= t_len/4)
    NCH = t_len // 16           # projection chunks of 512 slots

    nc = bass.Bass("TRN2", target_bir_lowering=False, debug=False)

    if USE_HOST_GATHER:
        xg = nc.dram_tensor("xg", [P, JT, E], bf16, kind="ExternalInput")
    else:
        emb = nc.dram_tensor("emb", [vocab, E], f32, kind="ExternalInput")
        idx = nc.dram_tensor("idx", [P, JT], mybir.dt.int32, kind="ExternalInput")
    w_in = {d: nc.dram_tensor(f"w_{d}", [P, KT, M8, P], bf16, kind="ExternalInput")
            for d in "fb"}
    u_in = {d: nc.dram_tensor(f"u_{d}", [P, KT, M8, P], bf16, kind="ExternalInput")
            for d in "fb"}
    b_in = {d: nc.dram_tensor(f"b_{d}", [1, M8, P], bf16, kind="ExternalInput")
            for d in "fb"}
    wd_in = nc.dram_tensor("wd", [P, 2 * KT, NTAGS], bf16, kind="ExternalInput")
    bd_in = nc.dram_tensor("bd", [P, 8 * NTAGS], f32, kind="ExternalInput")
    out = nc.dram_tensor("out", [P, JT, NTAGS], f32, kind="ExternalOutput")

    with tile.TileContext(nc) as tc, ExitStack() as ctx:
        cpool = ctx.enter_context(tc.tile_pool(name="const", bufs=1))
        xtpool = ctx.enter_context(tc.tile_pool(name="xt", bufs=1))
        xrpool = ctx.enter_context(tc.tile_pool(name="xr", bufs=2))
        gpool = ctx.enter_context(tc.tile_pool(name="g", bufs=GBUFS))
        hpool = ctx.enter_context(tc.tile_pool(name="h", bufs=3))
        spool = ctx.enter_context(tc.tile_pool(name="s", bufs=1))
        opool = ctx.enter_context(tc.tile_pool(name="o", bufs=4))
        tppool = ctx.enter_context(tc.tile_pool(name="tp", bufs=1, space="PSUM"))
        zpool = ctx.enter_context(tc.tile_pool(name="z", bufs=3, space="PSUM"))
        dpool = ctx.enter_context(tc.tile_pool(name="d", bufs=1, space="PSUM"))

        # ---- early gathers: first fwd and bwd chunks, issued before the
        # (larger) weight DMAs so transposes/projection start immediately ----
        early_xr = {}
        if USE_HOST_GATHER:
            NCHl = t_len // 16
            for ci in (0, NCHl - 1) if NCHl > 1 else (0,):
                exr = xrpool.tile([P, 4, E], bf16, tag="xr", name=f"exr{ci}")
                nc.sync.dma_start(exr[:], xg[:][:, 4 * ci:4 * ci + 4, :])
                early_xr[ci] = exr

        # ---- constant loads ----
        if not USE_HOST_GATHER:
            idx_sb = cpool.tile([P, JT], mybir.dt.int32)
            nc.sync.dma_start(idx_sb[:], idx[:])
        ident = cpool.tile([P, P], f32)
        make_identity(nc, ident[:])
        ident_bf = cpool.tile([P, P], bf16)
        nc.vector.tensor_copy(ident_bf[:], ident[:])
        # W before U: the first sigmoids only need W (+ the chunk transposes);
        # U is first consumed a full pair later.
        w_sb, u_sb, b_sb = {}, {}, {}
        for d in "fb":
            w_sb[d] = cpool.tile([P, KT, M8, P], bf16, tag=f"w{d}", name=f"wsb{d}")
            nc.sync.dma_start(w_sb[d][:], w_in[d][:])
            if not no_bias:
                b_sb[d] = cpool.tile([1, M8, P], bf16, tag=f"b{d}", name=f"bsb{d}")
                nc.sync.dma_start(b_sb[d][:], b_in[d][:])
        for d in "fb":
            u_sb[d] = cpool.tile([P, KT, M8, P], bf16, tag=f"u{d}", name=f"usb{d}")
            nc.sync.dma_start(u_sb[d][:], u_in[d][:])
        wd_sb = cpool.tile([P, 2 * KT, NTAGS], bf16)
        nc.sync.dma_start(wd_sb[:], wd_in[:])
        bd_sb = cpool.tile([P, 8, NTAGS], f32)
        nc.sync.dma_start(bd_sb[:], bd_in[:])
        ones_sb = cpool.tile([1, BS], bf16)
        nc.vector.memset(ones_sb[:], 1.0)

        xT = xtpool.tile([P, KT, SLOTS], bf16)

        # dense-psum bank doubles as PE-only scratch (disjoint column ranges):
        # scratch absorbs cross-engine waits so transpose matmuls (single hw
        # wait slot) never need two.  Four dense regions (dir x jj) so the f
        # and b dense calls never serialize on a WAR against the copies.
        dp_tile = dpool.tile([P, 128], f32)
        scr = dp_tile[0:32, 112:128].bitcast(bf16)
        nc.tensor.transpose(out=scr, in_=ident_bf[0:32, 0:32],
                            identity=ident_bf[0:32, 0:32])

        # ---- gather + transpose, just-in-time ----
        # The input projection W^T x is folded into each recurrence step's
        # PSUM accumulation group, so only the embedding gather and the
        # PE-transpose into xT remain as prep work.  Chunks 0/7 are prepared
        # in a short prelude; the rest is emitted as fine-grained work items
        # interleaved into the recurrence loop (one chunk of lookahead per
        # direction), so the in-order engine queues never bury a recurrence op
        # behind a long run of prep work.
        gathered = dict(early_xr)
        transposed = set()

        def gather_item(ci):
            if ci in gathered:
                return
            xr = xrpool.tile([P, 4, E], bf16, tag="xr")
            if USE_HOST_GATHER:
                nc.sync.dma_start(xr[:], xg[:][:, 4 * ci:4 * ci + 4, :])
            else:
                nc.gpsimd.indirect_dma_start(
                    out=xr[:], out_offset=None, in_=emb[:],
                    in_offset=bass.IndirectOffsetOnAxis(
                        ap=idx_sb[:, 4 * ci:4 * ci + 4], axis=0),
                )
            gathered[ci] = xr

        def transpose_item(ci, g, kt, use_act=False):
            xr = gathered[ci]
            if ci not in transposed:
                # PE-only scratch touch absorbs the cross-engine wait so the
                # real transposes keep a single hw wait slot.
                nc.tensor.transpose(out=scr, in_=xr[0:32, 0, 0:32],
                                    identity=ident_bf[0:32, 0:32])
                transposed.add(ci)
            gb = 4 * ci + g
            pt = tppool.tile([P, P], bf16, tag="tp", padded_shape=[None, 1024])
            nc.tensor.transpose(out=pt[:], in_=xr[:, g, kt * P:(kt + 1) * P],
                                identity=ident_bf[:])
            if use_act:
                nc.scalar.copy(out=xT[:, kt, gb * P:(gb + 1) * P], in_=pt[:])
            else:
                nc.vector.tensor_copy(out=xT[:, kt, gb * P:(gb + 1) * P], in_=pt[:])

        def chunk_items(ci, alternate=False):
            if ci in transposed:
                return []
            return ([(gather_item, ci)] +
                    [(transpose_item, ci, g, kt, alternate and (g + kt) % 2 == 0)
                     for g in range(4) for kt in range(KT)])

        # prelude: chunks 0 (fwd) and 7 (bwd); copies split across ACT and
        # DVE, since nothing else needs those engines yet and the first
        # sigmoid waits on chunk-0 landing in xT
        for it in chunk_items(0, alternate=True) + chunk_items(NCH - 1, alternate=True):
            it[0](*it[1:])
        transposed_pre = set(transposed)

        # lookahead worklists: during block k (pairs 16k..16k+15) prepare the
        # fwd chunk k+1 and bwd chunk NCH-2-k
        worklists = []
        for k in range(NCH - 1):
            merged = []
            for ci in (k + 1, NCH - 2 - k):
                if ci not in transposed_pre:
                    merged.append((gather_item, ci))
                    merged += [(transpose_item, ci, g, kt)
                               for g in range(4) for kt in range(KT)]
                    transposed_pre.add(ci)
            worklists.append(merged)
        worklists.append([])

        # ---- recurrence ----
        cell = {d: spool.tile([P, KT, BS], CDT, tag=f"c{d}", name=f"cell{d}") for d in "fb"}
        for d in "fb":
            nc.vector.memset(cell[d][:], 0.0)
        logits = {d: spool.tile([P, JT, NTAGS], f32, tag=f"lg{d}", name=f"logits{d}") for d in "fb"}
        hch = {"f": None, "b": None}
        hprev = {"f": None, "b": None}
        gates_cur = {"f": None, "b": None}
        zp_cur = {"f": None, "b": None}

        def half1(d, tau):
            """PE: z_t = W^T x_t (+ b) + U^T h_{t-1} in one PSUM group; ACT:
            all-gate sigmoid.  The x-projection matmuls are h-independent, so
            only the trailing U matmuls sit on the recurrence critical path."""
            t = tau if d == "f" else (t_len - 1 - tau)
            gates = gpool.tile([P, M8, BS], bf16, tag=f"g{d}")
            gates_cur[d] = gates
            # pad to a full 2KiB PSUM bank so the two in-flight zp buffers
            # never share a bank (concurrent PE-write + ACT-read of one bank
            # corrupts data)
            zp = zpool.tile([P, M8, BS], f32, tag=f"z{d}")
            # Exactly ONE start=True in the whole accumulation group: a
            # start-matmul clears the has_written bits for the entire PSUM
            # bank, so a second start would make later accumulates into other
            # regions overwrite instead of add.  With bits cleared once, each
            # region's first matmul overwrites and the rest accumulate.
            x_is_last = tau == 0
            first = True
            for m in range(M8):
                last_m = m == M8 - 1
                for kt in range(KT):
                    nc.tensor.matmul(
                        out=zp[:, m, :], lhsT=w_sb[d][:, kt, m, :],
                        rhs=xT[:, kt, BS * t:BS * (t + 1)],
                        start=first, skip_group_check=True,
                        stop=(x_is_last and no_bias and last_m and kt == KT - 1))
                    first = False
                if not no_bias:
                    nc.tensor.matmul(
                        out=zp[:, m, :], lhsT=b_sb[d][0:1, m, :],
                        rhs=ones_sb[0:1, :], start=False,
                        skip_group_check=True,
                        stop=(x_is_last and last_m))
            if tau != 0:
                tp = t + 1 if d == "b" else t - 1
                psl = tp % 8
                # h rotation happens in half2, so the previous step's h is
                # always still in hch here.
                hsrc = hch[d]
                # gate order is [i, f, g, o]: the (i,f,g) m-tiles 0..5 feed the
                # cell update, o (6..7) is only needed much later for h.  Run
                # the m0..5 matmuls first (k-tile-major so kt=0 only needs the
                # first half of the previous h write) and fire the main
                # sigmoid off just those 12; the o-matmuls and o-sigmoid
                # follow off the critical path.
                for kt in range(KT):
                    for m in range(6):
                        nc.tensor.matmul(
                            out=zp[:, m, :], lhsT=u_sb[d][:, kt, m, :],
                            rhs=hsrc[:, kt, BS * psl:BS * (psl + 1)],
                            start=False, skip_group_check=True, stop=False)
                nc.scalar.activation(gates[:, 0:6, :], zp[:, 0:6, :], SIG)
                for kt in range(KT):
                    for m in range(6, M8):
                        nc.tensor.matmul(
                            out=zp[:, m, :], lhsT=u_sb[d][:, kt, m, :],
                            rhs=hsrc[:, kt, BS * psl:BS * (psl + 1)],
                            start=False, skip_group_check=True,
                            stop=(m == M8 - 1 and kt == KT - 1))
                zp_cur[d] = zp
            else:
                nc.scalar.activation(gates[:, 0:8, :], zp[:, 0:8, :], SIG)
                zp_cur[d] = None

        def sig_o(d):
            """o-gate sigmoid: needed only for the h write, emitted so it
            slots into the ACT queue without delaying the tanh."""
            if zp_cur[d] is not None:
                nc.scalar.activation(gates_cur[d][:, 6:8, :],
                                     zp_cur[d][:, 6:8, :], SIG)

        def half2(d, tau):
            """DVE: fused cell update; ACT: tanh; DVE: h write.

            Gate cols (keras order): i=0:2, f=2:4, g=4:6, o=6:8 with the g
            z-columns pre-scaled x2 so sigmoid gives s = sigmoid(2*zg) and
            tanh(zg) = 2*s - 1.  Cell holds c/2 so the update is
            C = f*C + (s-0.5)*i, and the x2 folds into the tanh's free input
            scale.  STT1 first so its result latency hides under fc.
            """
            t = tau if d == "f" else (t_len - 1 - tau)
            sl = t % 8
            if tau % 8 == 0:
                hprev[d] = hch[d]
                hch[d] = hpool.tile([P, KT, 8 * BS], bf16, tag=f"h{d}", name=f"hch{d}")
            gates = gates_cur[d]
            t1 = gpool.tile([P, KT, BS], bf16, tag=f"t1{d}")
            nc.vector.scalar_tensor_tensor(
                out=t1[:], in0=gates[:, 4:6, :], scalar=0.5,
                in1=gates[:, 0:2, :], op0=mybir.AluOpType.subtract, op1=MUL)
            fc = gpool.tile([P, KT, BS], bf16, tag=f"fc{d}")
            nc.vector.tensor_tensor(out=fc[:], in0=gates[:, 2:4, :],
                                    in1=cell[d][:], op=MUL)
            nc.vector.tensor_tensor(out=cell[d][:], in0=fc[:], in1=t1[:],
                                    op=ADD)
            tct = gpool.tile([P, KT, BS], bf16, tag=f"tc{d}")
            nc.scalar.activation(tct[:], cell[d][:], TANH, scale=2.0)
            # split the h write by k-tile so the next step's U matmuls for
            # k-tile 0 can launch off the first write's semaphore
            for kt in range(KT):
                nc.vector.tensor_tensor(
                    out=hch[d][:, kt:kt + 1, BS * sl:BS * (sl + 1)],
                    in0=gates[:, 6 + kt:7 + kt, :], in1=tct[:, kt:kt + 1, :],
                    op=MUL)

        def dense(d, k):
            for jj in range(2):
                j = (2 * k + jj) if d == "f" else ((JT - 2) - 2 * k + jj)
                r = jj + (0 if d == "f" else 2)
                dp = dp_tile[:, NTAGS * r:NTAGS * (r + 1)]
                for kt in range(KT):
                    ktw = kt + (0 if d == "f" else KT)
                    nc.tensor.matmul(out=dp,
                                     lhsT=hch[d][:, kt, 128 * jj:128 * (jj + 1)],
                                     rhs=wd_sb[:, ktw, :],
                                     start=(kt == 0), stop=(kt == KT - 1))
                nc.vector.tensor_copy(out=logits[d][:, j, :], in_=dp)

        def softmax_block(j0, jn):
            """bias + softmax + store for j-tiles [j0, j0+jn).  Exp is safe
            unshifted: |logits| < ~6."""
            tmp = opool.tile([P, 8, NTAGS], f32, tag="sm")
            nc.vector.tensor_tensor(out=tmp[:, 0:jn, :],
                                    in0=logits["f"][:, j0:j0 + jn, :],
                                    in1=logits["b"][:, j0:j0 + jn, :], op=ADD)
            nc.vector.tensor_tensor(out=tmp[:, 0:jn, :], in0=tmp[:, 0:jn, :],
                                    in1=bd_sb[:, 0:jn, :], op=ADD)
            nc.scalar.activation(tmp[:, 0:jn, :], tmp[:, 0:jn, :], EXP)
            sm = opool.tile([P, 8, 1], f32, tag="smr")
            nc.vector.tensor_reduce(out=sm[:, 0:jn, :], in_=tmp[:, 0:jn, :],
                                    axis=mybir.AxisListType.X, op=ADD)
            rc = opool.tile([P, 8, 1], f32, tag="rc")
            nc.vector.reciprocal(out=rc[:, 0:jn, :], in_=sm[:, 0:jn, :])
            ost = opool.tile([P, 8, NTAGS], f32, tag="ost")
            nc.vector.tensor_tensor(out=ost[:, 0:jn, :], in0=tmp[:, 0:jn, :],
                                    in1=rc[:, 0:jn, :].to_broadcast([P, jn, NTAGS]),
                                    op=MUL)
            nc.sync.dma_start(out[:][:, j0:j0 + jn, :], ost[:, 0:jn, :])

        # softmax emission is deferred two pairs past the dense that completes
        # a j-pair, so its ops are long-ready when they reach the in-order
        # queues and never head-block the recurrence chain
        pending_sm = []
        for tau in range(t_len):
            half1("f", tau)
            half1("b", tau)
            for at, j0 in list(pending_sm):
                if at <= tau:
                    softmax_block(j0, 2)
                    pending_sm.remove((at, j0))
            sig_o("f")
            half2("f", tau)
            sig_o("b")
            half2("b", tau)
            if tau % 8 == 7:
                k = tau // 8
                dense("f", k)
                dense("b", k)
                # a j-pair is complete once BOTH directions have covered it:
                # fwd covers (2k, 2k+1) now; bwd covered it at block t_len//8-1-k.
                # From the halfway point on, two j-pairs complete per block.
                if k >= t_len // 16:
                    pending_sm.append((tau + 2, 2 * k))
                    jb = (JT - 2) - 2 * k
                    if jb != 2 * k:
                        pending_sm.append((tau + 3, jb))
            # emit this block's share of next-chunk gather/transpose work
            k, ph = tau // 16, tau % 16
            wl = worklists[k]
            n0 = len(wl) * ph // 16
            n1 = len(wl) * (ph + 1) // 16
            for it in wl[n0:n1]:
                it[0](*it[1:])
        for at, j0 in pending_sm:
            softmax_block(j0, 2)

    _legalize_waits(nc)
    return nc


# gate order is the native keras [i, f, g, o]
def _gate_perm():
    return np.arange(4 * H)


def marshal_weights(Wf, Uf, bf, Wb, Ub, bb, Wd, bd):
    import ml_dtypes
    perm = _gate_perm()
    gscale = np.ones(4 * H, np.float32)
    gscale[2 * H:3 * H] = 2.0     # g-gate columns
    def wmar(W):
        Wp = np.asarray(W, np.float32)[:, perm] * gscale
        return np.ascontiguousarray(
            Wp.reshape(KT, P, M8, P).transpose(1, 0, 2, 3)).astype(ml_dtypes.bfloat16)
    def bmar(b):
        bp = np.asarray(b, np.float32)[perm] * gscale
        return np.ascontiguousarray(bp.reshape(1, M8, P)).astype(ml_dtypes.bfloat16)
    wd = np.ascontiguousarray(
        np.asarray(Wd, np.float32).reshape(2 * KT, P, NTAGS)).astype(ml_dtypes.bfloat16)
    # [P, 2KT, NTAGS] with wd[p, kt, n] = Wd[kt*128+p, n]
    wd = np.ascontiguousarray(wd.transpose(1, 0, 2))
    bdt = np.ascontiguousarray(np.broadcast_to(np.tile(np.asarray(bd, np.float32), 8)[None, :], (P, 8 * NTAGS)))
    return {
        "w_f": wmar(Wf), "u_f": wmar(Uf), "b_f": bmar(bf),
        "w_b": wmar(Wb), "u_b": wmar(Ub), "b_b": bmar(bb),
        "wd": wd, "bd": bdt,
    }


def marshal_tokens(tokens_core, t_len=T):
    """tokens_core [BS, t_len] -> idx [128, t_len/4] int32 with
    idx[p, j] = tokens[p % 32, 4*j + p // 32]  (slot s = seq + 32*t)."""
    tk = np.asarray(tokens_core, np.int64)
    jt = BS * t_len // P
    p = np.arange(P)
    j = np.arange(jt)
    tt = 4 * j[None, :] + (p[:, None] // BS)
    return tk[(p[:, None] % BS), tt].astype(np.int32)


def unmarshal_out(out_core, t_len=T):
    """[128, JT, 17] slot-tile layout -> [BS, t_len, 17]."""
    slots = out_core.transpose(1, 0, 2).reshape(BS * t_len, NTAGS)
    return slots.reshape(t_len, BS, NTAGS).transpose(1, 0, 2)


def marshal_x(emb32, tokens_core, t_len=T):
    """Gather emb rows into the device slot layout [128, JT, E] (bf16)."""
    import ml_dtypes
    idx = marshal_tokens(tokens_core, t_len)     # [128, JT] int32
    return np.ascontiguousarray(emb32[idx]).astype(ml_dtypes.bfloat16)


def kernel(tokens, emb, Wf, Uf, bf, Wb, Ub, bb, Wd, bd):
    from concourse.bass_utils import run_bass_kernel_spmd

    no_bias = bool(np.all(np.asarray(bf) == 0) and np.all(np.asarray(bb) == 0))
    key = ("nc", no_bias)
    if key not in _CACHE:
        _CACHE[key] = build_program(no_bias=no_bias)
    nc = _CACHE[key]

    weights = marshal_weights(Wf, Uf, bf, Wb, Ub, bb, Wd, bd)
    emb32 = np.ascontiguousarray(np.asarray(emb, np.float32))
    tokens = np.asarray(tokens)
    in_maps = []
    for c in range(NCORES):
        tk = tokens[BS * c:BS * (c + 1)]
        if USE_HOST_GATHER:
            m = {"xg": marshal_x(emb32, tk)}
        else:
            m = {"emb": emb32, "idx": marshal_tokens(tk)}
        m.update(weights)
        in_maps.append(m)
    res = run_bass_kernel_spmd(nc, in_maps, core_ids=list(range(NCORES)))
    outs = [unmarshal_out(res.results[c]["out"]) for c in range(NCORES)]
    return np.concatenate(outs, axis=0).astype(np.float32)

